# revision 125
# baseline (speedup 1.0000x reference)
"""CapsNet dynamic-routing layer on 8 Trainium2 NeuronCores (Bass/Tile).

reference math (per batch element b):
  u_hat[b,i,o,j] = sum_d W[i,o,j,d] * u[b,i,d]        (never materialized)
  bl = 0; for r in 0..2:
    c = softmax_o(bl); s[b,o,j] = sum_i c*u_hat; v = squash(s)
    if r < 2: bl += sum_j u_hat*v
  return v  [B, 10, 16]

Distribution: pure data parallel, batch 512 -> 64 per core x 8 cores;
weights replicated.  Per-core: b=64, i=1152=9*128, o=10, j=16, d=8.

Layouts:
  s-matmuls (m1): stationary cu chunk [(i,d)=128, b=64], streamed
    W_s[(i,d)chunk, (o,j)] -> PSUM s[b=64, (o,j)=160].  Streaming the
    16-wide W slice per o instead of the 64-wide batch cuts PE column
    cycles ~4x vs the W-stationary orientation.
  agreement (m2): per o-pair q=(2q,2q+1), ONE matmul per 512-chunk with a
    block-diagonal v2 [32, 128] lhsT (j=16 rows per o, b-halves in
    columns) -> G for both o's in one 512-col stream.  Then
    ug = G (.) ur on DVE/Act/Pool (split), d-fold tree on DVE.
  softmax runs in b-partition space (bl [128=(b,h), 5, 1152]); exp'd
    slices are DMA-transposed to i-partition space where the o-sum,
    reciprocal and u*(1/Z) fold happen once, so no per-o normalize pass.
"""
import sys

sys.path.insert(0, "/opt/trn_rl_repo")

import numpy as np
import ml_dtypes
from contextlib import ExitStack

from concourse import bacc, mybir, hw_specs
from concourse.tile import TileContext
from concourse.bass_utils import run_bass_kernel_spmd

BF16 = mybir.dt.bfloat16
F32 = mybir.dt.float32
AX = mybir.AxisListType
ALU = mybir.AluOpType
ACTF = mybir.ActivationFunctionType
bfnp = ml_dtypes.bfloat16

B = 64
I = 1152
T = 9
O = 10
J = 16
D = 8
EPS = 1e-06
N_CORES = 8
KFLAT = D * I          # 9216 (d-major flat for m2)
NCH = KFLAT // 512     # 18
NCHU = T * D           # 72 chunks of 128 on the (i,d) contraction

# engine assignment knobs (GPSIMD cannot touch PSUM on real HW, so every
# Pool multiply reads SBUF after an Act drain; DVE can read PSUM directly)
DIRECT_NNS = (2, 7, 10, 15)                  # single chunks: DVE mult straight from PSUM
DIRECT_PAIR = (16, 17)               # chunk pair: DVE mult straight from PSUM
DRAIN_PAIRS = ((0, 1), (3, 4), (5, 6), (8, 9), (11, 12), (13, 14), (16, 17))
DVE_PAIRS = frozenset({(0, 1), (11, 12)})      # drained pairs multiplied on DVE (rest Pool)
POOL_OS = frozenset({1, 3, 6, 8})    # cu multiplies done on GPSIMD

_cache = {}

# Route every activation through the one table set that has exp+ln+copy,
# so the ACT engine never reloads tables mid-kernel.
_KEEP_SET = "natural_log_exp_and_others"


def _patched_tables(arch):
    full = {k: set(v) for k, v in hw_specs.get_activation_tables(arch).items()}
    keep = full[_KEEP_SET]
    return {k: (v if k == _KEEP_SET else v - keep) for k, v in full.items()}


import os
if os.environ.get('ACT_PATCH', '1') == '1':
    bacc.get_activation_tables = _patched_tables


def _oslot(o):
    """o -> (pair q / bl slot, psum half h).  o = 2q+h for o<8; pair 4 = (8,9)."""
    if o < 8:
        return o // 2, o % 2
    return 4, o - 8


def build_nc():
    nc = bacc.Bacc()
    ws_d = nc.dram_tensor("ws", [128, NCHU, O * J], BF16, kind="ExternalInput")
    wba_d = nc.dram_tensor("wba", [128, KFLAT], BF16, kind="ExternalInput")
    wbb_d = nc.dram_tensor("wbb", [32, KFLAT], BF16, kind="ExternalInput")
    ui_d = nc.dram_tensor("ui", [128, T, D, B], BF16, kind="ExternalInput")
    ur_d = nc.dram_tensor("ur", [128, KFLAT], BF16, kind="ExternalInput")
    id64_d = nc.dram_tensor("id64", [B, B], BF16, kind="ExternalInput")
    vout_d = nc.dram_tensor("vout", [B, O, J], F32, kind="ExternalOutput")

    with TileContext(nc) as tc, ExitStack() as ctx:
        static = ctx.enter_context(tc.tile_pool(name="static", bufs=1))
        work = ctx.enter_context(tc.tile_pool(name="work", bufs=1))
        cupool = ctx.enter_context(tc.tile_pool(name="cup", bufs=2))
        ugpool = ctx.enter_context(tc.tile_pool(name="ugp", bufs=2))
        psS = ctx.enter_context(tc.tile_pool(name="psS", bufs=1, space="PSUM"))
        psT = ctx.enter_context(tc.tile_pool(name="psT", bufs=1, space="PSUM"))
        psM = ctx.enter_context(tc.tile_pool(name="psM", bufs=2, space="PSUM"))
        psN = ctx.enter_context(tc.tile_pool(name="psN", bufs=2, space="PSUM"))

        ws = static.tile([128, NCHU, O * J], BF16, name="ws")
        wba = static.tile([128, KFLAT], BF16, name="wba")
        wbb = static.tile([32, KFLAT], BF16, name="wbb")
        ui = static.tile([128, T, D, B], BF16, name="ui")
        ur = static.tile([128, KFLAT], BF16, name="ur")
        # split input loads across the two hwdge queues: SP carries what
        # iteration 0 needs (ws, ui), interleaved in it0's chunk order so
        # the PE can start after the first slice; Act carries the m2-side
        # tensors.
        for k in range(4):
            csl = slice(18 * k, 18 * k + 9)
            csl2 = slice(18 * k + 9, 18 * (k + 1))
            tsl = slice((T * k) // 4, (T * (k + 1)) // 4)
            nc.sync.dma_start(out=ws[:, csl, :], in_=ws_d[:, csl, :])
            nc.scalar.dma_start(out=ws[:, csl2, :], in_=ws_d[:, csl2, :])
            if k % 2 == 0:
                nc.sync.dma_start(out=ui[:, tsl, :, :], in_=ui_d[:, tsl, :, :])
            else:
                nc.scalar.dma_start(out=ui[:, tsl, :, :],
                                    in_=ui_d[:, tsl, :, :])
        id64 = static.tile([B, B], BF16, name="id64")
        nc.sync.dma_start(out=id64, in_=id64_d[:, :])
        nc.gpsimd.dma_start(out=wba[:, 0:4608], in_=wba_d[:, 0:4608])
        nc.gpsimd.dma_start(out=ur[:, 0:4608], in_=ur_d[:, 0:4608])
        nc.gpsimd.dma_start(out=wba[:, 4608:KFLAT], in_=wba_d[:, 4608:KFLAT])
        nc.sync.dma_start(out=ur[:, 4608:KFLAT], in_=ur_d[:, 4608:KFLAT])
        nc.sync.dma_start(out=wbb, in_=wbb_d[:, :])

        bl = work.tile([128, 5, I], F32, name="bl")
        e = work.tile([128, 5, I], BF16, name="e")
        et = work.tile([128, T, O, B], BF16, name="et")
        zt = work.tile([128, T, B], BF16, name="zt")
        ztp = work.tile([128, T, B], BF16, name="ztp")
        ztf = work.tile([128, T, B], F32, name="ztf")
        rz = work.tile([128, T, B], F32, name="rz")
        rzb = work.tile([128, T, B], BF16, name="rzb")
        uz = work.tile([128, T, D, B], BF16, name="uz")
        s_sb = work.tile([B, O, J], F32, name="s_sb")
        s2 = work.tile([B, O, J], F32, name="s2")
        sq = work.tile([B, O], F32, name="sq")
        t1 = work.tile([B, O], F32, name="t1")
        t2 = work.tile([B, O], F32, name="t2")
        den = work.tile([B, O], F32, name="den")
        rec = work.tile([B, O], F32, name="rec")
        wsc = work.tile([B, O], F32, name="wsc")
        v_sb = work.tile([B, O, J], F32, name="v_sb")
        # vz panels: b-partition staging for the block-diag v2 transposes.
        # vz0 cols 32q..32q+16 = v[b, 2q, :]; vz1 cols 32q+16..32q+32 =
        # v[b, 2q+1, :]; vzb0 cols 0:16 = v[b, 8, :]; vzb1 cols 16:32 =
        # v[b, 9, :]; all other columns stay zero forever.
        vz0 = work.tile([B, 128], BF16, name="vz0")
        vz1 = work.tile([B, 128], BF16, name="vz1")
        vzb0 = work.tile([B, 128], BF16, name="vzb0")
        vzb1 = work.tile([B, 128], BF16, name="vzb1")
        v2a = work.tile([128, 128], BF16, name="v2a")
        v2b = work.tile([128, 128], BF16, name="v2b")
        eps1 = work.tile([B, 1], F32, name="eps1")
        nc.vector.memset(eps1, EPS)
        for z in (vz0, vz1, vzb0, vzb1):
            nc.vector.memset(z, 0.0)

        # ------------- m1: s accumulation -------------
        def m1_it0():
            """s~ = sum_(i,d) W u for all 10 o at once (c=0.1 applied in
            squash).  Two alternating PSUM banks so consecutive matmuls
            pipeline past the PSUM access latency."""
            ps = psS.tile([B, O * J], F32, name="sps", tag="sps")
            for ch in range(NCHU):
                t, d = divmod(ch, D)
                nc.tensor.matmul(
                    ps, ui[:, t, d, :], ws[:, ch, :],
                    start=(ch == 0), stop=(ch == NCHU - 1),
                )
            return ps

        TBLK = ((0, 3), (3, 6), (6, 9))

        def m1_iter():
            """s_o = sum_(i,d) W_o (c_o*u), per-o cu stationary."""
            ps = psS.tile([B, O * J], F32, name="sps", tag="sps")
            for o in range(O):
                cu = cupool.tile([128, T, D, B], BF16, name="cu", tag="cu")
                if o in POOL_OS:
                    # GPSIMD stt is limited to 2D/3D APs: go per-t slice.
                    for t in range(T):
                        ebc = et[:, t, o, :].unsqueeze(1).broadcast_to(
                            [128, D, B])
                        nc.gpsimd.tensor_tensor(cu[:, t, :, :], ebc,
                                                uz[:, t, :, :], op=ALU.mult)
                else:
                    for t0, t1 in TBLK:
                        tb = slice(t0, t1)
                        ebc = et[:, tb, o, :].unsqueeze(2).broadcast_to(
                            [128, t1 - t0, D, B])
                        nc.vector.tensor_tensor(cu[:, tb, :, :], ebc,
                                                uz[:, tb, :, :], op=ALU.mult)
                for ch in range(NCHU):
                    t, d = divmod(ch, D)
                    nc.tensor.matmul(
                        ps[:, J * o : J * (o + 1)],
                        cu[:, t, d, :],
                        ws[:, ch, J * o : J * (o + 1)],
                        start=(ch == 0), stop=(ch == NCHU - 1),
                    )
            return ps

        # ------------- squash + v2 build -------------
        def squash(ps, scale, it):
            sf = s_sb.rearrange("b o j -> b (o j)")
            s2f = s2.rearrange("b o j -> b (o j)")
            nc.scalar.mul(sf, ps, scale)
            nc.vector.tensor_tensor(s2f, sf, sf, op=ALU.mult)
            nc.vector.tensor_reduce(sq, s2, axis=AX.X, op=ALU.add)
            nc.scalar.activation(t1, sq, ACTF.Ln, bias=eps1)
            nc.scalar.activation(t2, t1, ACTF.Exp, scale=0.5)  # sqrt(sq+eps)
            nc.vector.tensor_scalar_add(den, sq, 1.0)
            nc.vector.tensor_tensor(den, den, t2, op=ALU.mult)
            nc.vector.reciprocal_approx_accurate(rec, den, t1)
            nc.vector.tensor_tensor(wsc, sq, rec, op=ALU.mult)
            nc.vector.tensor_tensor(
                v_sb, s_sb, wsc.unsqueeze(2).broadcast_to([B, O, J]),
                op=ALU.mult)
            if it == 2:
                nc.sync.dma_start(out=vout_d[:, :, :], in_=v_sb)
                return
            vz0v = vz0.rearrange("b (q c) -> b q c", q=4)
            vz1v = vz1.rearrange("b (q c) -> b q c", q=4)
            nc.vector.tensor_copy(vz0v[:, :, 0:16], v_sb[:, 0:8:2, :])
            nc.vector.tensor_copy(vz1v[:, :, 16:32], v_sb[:, 1:9:2, :])
            nc.vector.tensor_copy(vzb0[:, 0:16], v_sb[:, 8, :])
            nc.vector.tensor_copy(vzb1[:, 16:32], v_sb[:, 9, :])
            # transpose the vz panels onto v2 via the PE (identity matmul)
            # instead of DMA transposes: no DMA queue contention and no
            # dge completion delay on the round-boundary critical path.
            pt = psT.tile([128, 256], F32, name="v2t", tag="v2t")
            nc.tensor.matmul(pt[:, 0:64], vz0, id64, start=True, stop=True)
            nc.tensor.matmul(pt[:, 64:128], vz1, id64, start=True, stop=True)
            nc.tensor.matmul(pt[0:32, 128:192], vzb0[:, 0:32], id64,
                             start=True, stop=True)
            nc.tensor.matmul(pt[0:32, 192:256], vzb1[:, 0:32], id64,
                             start=True, stop=True)
            nc.scalar.copy(v2a, pt[:, 0:128])
            nc.scalar.copy(v2b[0:32, :], pt[0:32, 128:256])

        # ------------- m2: agreement -> bl -------------
        def m2(it):
            for q in range(5):
                ug = ugpool.tile([128, KFLAT], BF16, name="ug", tag="ug")

                def mm(dst, nn):
                    csl = slice(512 * nn, 512 * (nn + 1))
                    if q < 4:
                        nc.tensor.matmul(
                            dst, v2a[32 * q : 32 * q + 32, :],
                            wba[32 * q : 32 * q + 32, csl],
                            start=True, stop=True,
                            tile_position=(32 * q, 0),
                        )
                    else:
                        nc.tensor.matmul(
                            dst, v2b[0:32, :], wbb[:, csl],
                            start=True, stop=True,
                            tile_position=(0, 0),
                        )

                def drain_pair(n1, n2):
                    pp = psM.tile([128, 1024], F32, name="m2p", tag="m2p")
                    mm(pp[:, 0:512], n1)
                    mm(pp[:, 512:1024], n2)
                    csl = slice(512 * n1, 512 * (n2 + 1))
                    nc.scalar.copy(ug[:, csl], pp)
                    if (n1, n2) in DVE_PAIRS:
                        nc.vector.tensor_tensor(ug[:, csl], ug[:, csl],
                                                ur[:, csl], op=ALU.mult)
                    else:
                        nc.gpsimd.tensor_tensor(ug[:, csl], ug[:, csl],
                                                ur[:, csl], op=ALU.mult)

                def direct(nn):
                    ps = psN.tile([128, 512], F32, name="m2d", tag="m2d")
                    mm(ps, nn)
                    csl = slice(512 * nn, 512 * (nn + 1))
                    nc.vector.tensor_tensor(ug[:, csl], ps, ur[:, csl],
                                            op=ALU.mult)

                for nn in DIRECT_NNS:
                    direct(nn)
                for n1, n2 in DRAIN_PAIRS:
                    drain_pair(n1, n2)
                # d-fold tree (d-major flat: level k folds d, d+4 / d+2 / d+1)
                # level 1 on DVE (bf16 2x rate), the f32-tainted tail on Pool.
                # The last pair (q=4) gates the next softmax, so its levels
                # are split DVE || Pool to cut the boundary latency.
                nc.vector.tensor_tensor(ug[:, 0:2048], ug[:, 0:2048],
                                        ug[:, 4608:6656], op=ALU.add)
                nc.vector.tensor_tensor(ug[:, 2048:4608], ug[:, 2048:4608],
                                        ug[:, 6656:9216], op=ALU.add)
                nc.vector.tensor_tensor(ug[:, 0:2304], ug[:, 0:2304],
                                        ug[:, 2304:4608], op=ALU.add)
                if it == 0:
                    nc.gpsimd.tensor_tensor(bl[:, q, :], ug[:, 0:I],
                                            ug[:, I : 2 * I], op=ALU.add)
                else:
                    tmp = ug[:, 2304 : 2304 + I]
                    nc.gpsimd.tensor_tensor(tmp, ug[:, 0:I],
                                            ug[:, I : 2 * I], op=ALU.add)
                    nc.gpsimd.tensor_tensor(bl[:, q, :], bl[:, q, :], tmp,
                                            op=ALU.add)

        # ------------- softmax (i-space) + u/Z fold -------------
        def softmax():
            # slot 4 (the last pair) is exp'd in t-thirds so the t-blocked
            # tail below can start before the whole slot is done.
            for p in range(4):
                nc.scalar.activation(e[:, p, :], bl[:, p, :], ACTF.Exp)
            for t0, t1 in TBLK:
                nc.scalar.activation(e[:, 4, 128 * t0 : 128 * t1],
                                     bl[:, 4, 128 * t0 : 128 * t1], ACTF.Exp)
            for o in range(O):
                sl, h = _oslot(o)
                for t in range(T):
                    nc.sync.dma_start_transpose(
                        out=et[:, t, o, :],
                        in_=e[64 * h : 64 * h + 64, sl,
                              128 * t : 128 * (t + 1)],
                    )
            # Per t-block: DVE sums o={0,2,4,6,8,9} (late pairs last), Pool
            # sums o={1,3,5,7}, DVE merges, recip, 1/Z fold into u.
            for t0, t1 in TBLK:
                tb = slice(t0, t1)
                nc.vector.tensor_tensor(zt[:, tb, :], et[:, tb, 0, :],
                                        et[:, tb, 2, :], op=ALU.add)
                for o in (4, 6, 8, 9):
                    nc.vector.tensor_tensor(zt[:, tb, :], zt[:, tb, :],
                                            et[:, tb, o, :], op=ALU.add)
                nc.gpsimd.tensor_tensor(ztp[:, tb, :], et[:, tb, 1, :],
                                        et[:, tb, 3, :], op=ALU.add)
                for o in (5, 7):
                    nc.gpsimd.tensor_tensor(ztp[:, tb, :], ztp[:, tb, :],
                                            et[:, tb, o, :], op=ALU.add)
                nc.vector.tensor_tensor(ztf[:, tb, :], zt[:, tb, :],
                                        ztp[:, tb, :], op=ALU.add)
                nc.vector.reciprocal_approx_fast(rz[:, tb, :], ztf[:, tb, :])
                nc.vector.tensor_copy(rzb[:, tb, :], rz[:, tb, :])
                if t0 == 3:
                    for t in range(t0, t1):
                        nc.gpsimd.tensor_tensor(
                            uz[:, t, :, :], ui[:, t, :, :],
                            rzb[:, t, :].unsqueeze(1).broadcast_to(
                                [128, D, B]),
                            op=ALU.mult)
                else:
                    nc.vector.tensor_tensor(
                        uz[:, tb, :, :], ui[:, tb, :, :],
                        rzb[:, tb, :].unsqueeze(2).broadcast_to(
                            [128, t1 - t0, D, B]),
                        op=ALU.mult)

        # ========================= flow =========================
        ps0 = m1_it0()
        squash(ps0, 0.1, 0)
        m2(0)
        softmax()
        ps1 = m1_iter()
        squash(ps1, 1.0, 1)
        m2(1)
        softmax()
        ps2 = m1_iter()
        squash(ps2, 1.0, 2)

    nc.finalize()
    return nc


def _host_prep(u, weights):
    """Per-core input maps. u [512,1152,8] f32, weights [1152,10,16,8] f32."""
    W = np.asarray(weights, dtype=np.float32)
    u = np.asarray(u, dtype=np.float32)
    # W_s[p, t*8+d, o*16+j] = W[t*128+p, o, j, d]
    ws = np.ascontiguousarray(
        W.reshape(T, 128, O, J, D).transpose(1, 0, 4, 2, 3)
    ).reshape(128, NCHU, O * J).astype(bfnp)
    # wb rows 32q+16h+j = W[o=2q+h][j, d-major flat]
    wt = W.transpose(1, 2, 3, 0).reshape(O, J, KFLAT)  # [o, j, (d,i)]
    wba = np.zeros((128, KFLAT), dtype=bfnp)
    for o in range(8):
        q, h = o // 2, o % 2
        wba[32 * q + 16 * h : 32 * q + 16 * h + 16, :] = wt[o].astype(bfnp)
    wbb = np.zeros((32, KFLAT), dtype=bfnp)
    for o in (8, 9):
        h = o - 8
        wbb[16 * h : 16 * h + 16, :] = wt[o].astype(bfnp)

    base = {"ws": ws, "wba": wba, "wbb": wbb,
            "id64": np.eye(B, dtype=bfnp)}
    in_maps = []
    for c in range(N_CORES):
        uc = u[c * B : (c + 1) * B]  # [64, 1152, 8]
        ui = np.ascontiguousarray(
            uc.reshape(B, T, 128, D).transpose(2, 1, 3, 0)
        ).astype(bfnp)  # [128, T, D, B]
        urh = np.ascontiguousarray(uc.transpose(0, 2, 1)).reshape(B, KFLAT)
        ur = np.concatenate([urh, urh], axis=0).astype(bfnp)  # [128, KFLAT]
        in_maps.append({**base, "ui": ui, "ur": ur})
    return in_maps


def kernel(u, weights):
    if "nc" not in _cache:
        _cache["nc"] = build_nc()
    nc = _cache["nc"]
    in_maps = _host_prep(u, weights)
    res = run_bass_kernel_spmd(nc, in_maps, core_ids=list(range(N_CORES)))
    out = np.concatenate([res.results[c]["vout"] for c in range(N_CORES)], axis=0)
    return out.astype(np.float32)


if __name__ == "__main__":
    rng = np.random.default_rng(0)
    u = rng.standard_normal((512, 1152, 8), dtype=np.float32)
    w = (rng.standard_normal((1152, 10, 16, 8)) * 0.1).astype(np.float32)
    v = kernel(u, w)
    print("out", v.shape, v.dtype, np.abs(v).max())
